# revision 44
# baseline (speedup 1.0000x reference)
"""Trainium2 Bass kernel for nn_Expression_Independent_AU_Loss.

Loss over pred [B=4194304, C=16] (target is unused by the reference):
  pos[c]  = sum_r pred[r,c] * (pred[r,c] >= 0.5) / B
  neg[c]  = sum_r pred[r,c] * (pred[r,c] <  0.5) / B   (= total[c]/B - pos[c])
  pp[i,j] = sum_r y[r,i]*y[r,j] / B   with y = pred * (pred >= 0.5)
followed by a tiny clamp/combine over 14 column pairs (11 distinct columns).

v2 strategy (fp8 + DoubleRow, data-parallel over batch, 8 cores):
  - Host: permute columns so the 8 distinct pair-j columns sit at positions
    0..7 and the 3 i-only columns at 8..10, cast to fp8e4m3 with exact-mask
    repair (values that round up across 0.5 get nudged to 0.46875, the
    largest fp8 below 0.5). DMA halves again vs fp16: 8 MiB/core.
  - Each core views its shard as [128, 65536] fp8; a row-group of 16
    consecutive elements is one full (permuted) row.
  - Per tile: DVE+Pool split-compute y = (x>=0.5)*x for the 8 j-columns into
    a 16-slot z-tile (slot 0 = constant 1). The Act engine computes
    r = Relu(x-0.5) and s = Sign(x-0.484375) for the 3 i-only columns
    (slots 9..11 / 12..14): y = r + s/4 + 1/4 EXACTLY for fp8 inputs, so
    those columns never need the elementwise mask.
  - TensorE (fp8 DoubleRow, 2x MACs, pairs = tile halves):
      gram psum[112,72] += Zst^T @ Zmv  per 8-group chunk
        Zst = slots 1..14 (8 y + 3 r + 3 s), Zmv = slots 0..8 (ones + 8 y)
      colsum psum[1,512] += 1^T @ X     on the raw fp8 tile (total, unmasked)
  - Host combines the tiny per-core partials: reconstructs pp for pairs with
    i-only stationary via pp = G_ry + G_sy/4 + pos_j/4, then clamp/combine.
"""

import numpy as np

_B, _C = 4194304, 16
_NCORES = 8
_FD_TOTAL = _B // _NCORES * _C // 128  # 65536 fp8 per partition per core
_FD_TILE = 4096

_POS_PAIRS = [(0, 1), (2, 5), (2, 6), (5, 6), (4, 8), (6, 11), (9, 11), (9, 14), (11, 14), (13, 14)]
_NEG_PAIRS = [(1, 4), (1, 5), (8, 9), (8, 11)]

# column permutation: j-set first (moving side), then i-only, then unused
_JCOLS = [1, 4, 5, 6, 8, 9, 11, 14]
_ICOLS = [0, 2, 13]
_PERM = _JCOLS + _ICOLS + [3, 7, 10, 12, 15]
_PC = {c: i for i, c in enumerate(_PERM)}  # orig col -> permuted position

_NMV = 9    # moving slots 0..8
_NG = 8     # row-groups per chunk
_N_YCOLS = 8  # cols masked on DVE (rest via act r/s) — tuned empirically
_SHIP_FLAGS = {}  # extra _build flags for the shipped configuration half

_built = {}


def _build(fd_total, fd_tile, repeat=1, xin_bufs=3, mask_frac=1.0,
           n_ycols=8, dve_split=False, relu_on_dve=False, alt_dma=False,
           z_bufs=2, do_mask=True, do_act=True, do_gram=True, do_xsum=True,
           do_dve=True, do_pool=False, do_mm=True):
    """Build + compile the SPMD Bass program for one core shard
    [128, fd_total] fp8e4m3 (flat, contiguous per partition).

    repeat>1 re-runs the whole pass over the same input (differential HW
    timing); partials come out scaled by `repeat`. do_* flags ablate stages
    (timing only). mask_frac = fraction of j-column mask work on DVE
    (rest on Pool/GpSimd).
    """
    import concourse.bass as bass  # noqa: F401
    import concourse.tile as tile
    from concourse import bacc, mybir

    f32 = mybir.dt.float32
    f8 = mybir.dt.float8e4
    DR = mybir.MatmulPerfMode.DoubleRow
    AF = mybir.ActivationFunctionType

    n_tiles = fd_total // fd_tile
    assert fd_total % fd_tile == 0 and fd_tile % 1024 == 0
    n_y = n_ycols               # cols masked on DVE (slots 1..n_y)
    n_rs = 11 - n_y             # i-only cols via act r/s pairs
    n_st = n_y + 2 * n_rs       # stationary slots (1..n_st)
    if n_rs == 0:
        do_act = False

    nc = bacc.Bacc("TRN2", target_bir_lowering=False, debug=False)
    x = nc.dram_tensor("x", [128 * fd_total], f8, kind="ExternalInput").ap()
    gram_out = nc.dram_tensor(
        "gram", [8 * n_st, 72], f32, kind="ExternalOutput").ap()
    colsum_out = nc.dram_tensor("colsum", [1, 512], f32, kind="ExternalOutput").ap()

    G = fd_tile // 16           # row-groups per tile
    NC = G // _NG               # 8-group chunks per tile (chunk-major z)
    n_gram = NC // 2            # DR gram matmuls per tile (chunk c + c+NC/2)
    n_x512 = fd_tile // 1024    # xsum DR matmuls per tile (512 out cols each)

    with tile.TileContext(nc) as tc:
        with (
            tc.tile_pool(name="xin", bufs=xin_bufs) as xin_pool,
            tc.tile_pool(name="zp", bufs=1) as z_pool,
            tc.tile_pool(name="cst", bufs=1) as cst_pool,
            tc.tile_pool(name="outs", bufs=1) as out_pool,
            tc.tile_pool(name="psum", bufs=1, space="PSUM") as psum_pool,
        ):
            if do_xsum:
                # dual-fp8 ldweights needs pair stride >= 64: [128,128] ones
                ones_x = cst_pool.tile([128, 128], f8, tag="onesx")
                nc.vector.memset(ones_x[:], 1.0)
                st_ones = ones_x[:].rearrange("p (two m) -> p two m", two=2)[:, :, 0:1]
                psum_b = psum_pool.tile([128, 512], f32, tag="pb")

            if do_gram:
                psum_a = psum_pool.tile([8 * n_st, 72], f32, tag="pa")
                zbufs = []
                for zi in range(z_bufs):
                    # chunk-major, slot-major: z[p, chunk, slot(16), group(8)]
                    zt = z_pool.tile([128, G * 16], f8, tag=f"z{zi}")
                    z3 = zt[:].rearrange("p (c w) -> p c w", w=128)
                    nc.vector.memset(z3[:, :, 0:8], 1.0)  # slot 0 = ones
                    zbufs.append(zt)
                if do_act:
                    bias_r = cst_pool.tile([128, 1], f32, tag="br")
                    bias_s = cst_pool.tile([128, 1], f32, tag="bs")
                    nc.vector.memset(bias_r[:], -0.5)
                    nc.vector.memset(bias_s[:], -0.484375)

            # chunk split for the DVE/Pool mask work
            s_dve = int(round(mask_frac * NC))

            for r in range(repeat):
                first_r, last_r = r == 0, r == repeat - 1
                off = 0
                for t in range(n_tiles):
                    first_t, last_t = t == 0, t == n_tiles - 1
                    xt = xin_pool.tile([128, fd_tile], f8, tag="x")
                    src = x[128 * off: 128 * (off + fd_tile)].rearrange(
                        "(p f) -> p f", p=128)
                    dma_eng = nc.gpsimd if (alt_dma and t % 2) else nc.sync
                    dma_eng.dma_start(xt[:], src)

                    # x and z share chunk-major blocks of 128: x block =
                    # [col(16) x group(8)] (host pre-transposed), z block =
                    # [slot(16) x group(8)]; all elementwise ops are 3D
                    # stride-1 views [p, chunk, span]
                    xc = xt[:].rearrange("p (c w) -> p c w", w=128)
                    if do_gram:
                        zt = zbufs[t % z_bufs]
                        zc3 = zt[:].rearrange("p (c w) -> p c w", w=128)
                        if do_mask:
                            # y = (x >= 0.5) * x for the first n_y cols
                            # (slots 1..n_y <- cols 0..n_y-1, same group order)
                            if do_dve and s_dve > 0:
                                spans = ([(0, 8)] if n_y == 8 else
                                         [(0, 8), (8, n_y)] if dve_split else
                                         [(0, n_y)])
                                for c0, c1 in spans:
                                    nc.vector.scalar_tensor_tensor(
                                        zc3[:, 0:s_dve, 8 + 8 * c0:8 + 8 * c1],
                                        xc[:, 0:s_dve, 8 * c0:8 * c1],
                                        0.5, xc[:, 0:s_dve, 8 * c0:8 * c1],
                                        op0=mybir.AluOpType.is_ge,
                                        op1=mybir.AluOpType.mult)
                            if do_pool and s_dve < NC:
                                # Pool lacks TensorScalarPtr: two-stage
                                # m = (x>=0.5) in place, then y = m*x
                                nc.gpsimd.tensor_scalar(
                                    zc3[:, s_dve:NC, 8:8 + 8 * n_y],
                                    xc[:, s_dve:NC, 0:8 * n_y],
                                    0.5, None, op0=mybir.AluOpType.is_ge)
                                nc.gpsimd.tensor_tensor(
                                    zc3[:, s_dve:NC, 8:8 + 8 * n_y],
                                    zc3[:, s_dve:NC, 8:8 + 8 * n_y],
                                    xc[:, s_dve:NC, 0:8 * n_y],
                                    op=mybir.AluOpType.mult)
                        if do_act:
                            # r = Relu(x - 0.5), s = Sign(x - 0.484375) for
                            # the n_rs i-only cols; y = r + s/4 + 1/4 exactly
                            ro = 8 * (1 + n_y)
                            so = ro + 8 * n_rs
                            if relu_on_dve:
                                # r = max(x - 0.5, 0) as a DVE tensor_scalar
                                nc.vector.tensor_scalar(
                                    zc3[:, :, ro:ro + 8 * n_rs],
                                    xc[:, :, 8 * n_y:88],
                                    0.5, 0.0,
                                    op0=mybir.AluOpType.subtract,
                                    op1=mybir.AluOpType.max)
                            else:
                                nc.scalar.activation(
                                    zc3[:, :, ro:ro + 8 * n_rs],
                                    xc[:, :, 8 * n_y:88],
                                    AF.Relu, bias=bias_r[:], scale=1.0)
                            nc.scalar.activation(
                                zc3[:, :, so:so + 8 * n_rs],
                                xc[:, :, 8 * n_y:88],
                                AF.Sign, bias=bias_s[:], scale=1.0)

                        # DR gram: pair chunk k with chunk k + NC/2; both
                        # operands contiguous within a chunk block
                        if do_mm:
                            zc = zt[:].rearrange("p (h rest) -> p h rest", h=2)
                            for k in range(n_gram):
                                st = zc[:, :, 128 * k + 8:
                                        128 * k + 8 + 8 * n_st]
                                mv = zc[:, :, 128 * k: 128 * k + 72]
                                nc.tensor.matmul(
                                    psum_a[:, :], st, mv,
                                    start=(first_r and first_t and k == 0),
                                    stop=(last_r and last_t and k == n_gram - 1),
                                    perf_mode=DR)

                    if do_xsum:
                        x2 = xt[:].rearrange("p (h f) -> p h f", h=2)
                        for j in range(n_x512):
                            nc.tensor.matmul(
                                psum_b[0:1, 0:512],
                                st_ones,
                                x2[:, :, 512 * j: 512 * (j + 1)],
                                start=(first_r and first_t and j == 0),
                                stop=(last_r and last_t and j == n_x512 - 1),
                                perf_mode=DR)
                    off += fd_tile

            if do_xsum:
                out_b = out_pool.tile([1, 512], f32, tag="ob")
                nc.vector.tensor_copy(out_b[:], psum_b[0:1, :])
                nc.sync.dma_start(colsum_out[:], out_b[:])
            if do_gram and do_mm:
                out_a = out_pool.tile([8 * n_st, 72], f32, tag="oa")
                nc.vector.tensor_copy(out_a[:], psum_a[:])
                nc.sync.dma_start(gram_out[:], out_a[:])

    nc.compile()
    return nc


def _get_nc(fd_total, fd_tile, repeat=1, xin_bufs=3, **flags):
    key = (fd_total, fd_tile, repeat, xin_bufs, tuple(sorted(flags.items())))
    if key not in _built:
        _built[key] = _build(fd_total, fd_tile, repeat, xin_bufs, **flags)
    return _built[key]


def prep_input(pred, fd_total=_FD_TOTAL):
    """fp8e4m3 staging: permute columns (j-set first), round-nearest cast,
    then nudge elements that rounded up across the 0.5 threshold down to
    0.46875 (largest fp8 < 0.5). Mask becomes exact; value rounding stays
    unbiased apart from the small nudge population (~1.6% of rows)."""
    from concourse import mybir
    np8 = mybir.dt.np(mybir.dt.float8e4)
    p32 = np.ascontiguousarray(pred, dtype=np.float32)[:, _PERM]
    p8 = p32.astype(np8)
    flipped = (p8.astype(np.float32) >= 0.5) & (p32 < 0.5)
    p8[flipped] = np8(0.46875)
    # per-partition chunk blocks of 8 rows transposed to [col(16), group(8)]
    # so every device elementwise op is a 3D stride-1 view
    rows_pp = fd_total // 16
    a = p8.reshape(_NCORES, 128, rows_pp // 8, 8, 16)
    a = np.ascontiguousarray(a.transpose(0, 1, 2, 4, 3))
    return a.reshape(_NCORES, 128 * fd_total)


def run_cores(pred, fd_total=_FD_TOTAL, fd_tile=_FD_TILE, trace=False, **flags):
    """Run the per-core program over all 8 shards; returns raw results."""
    from concourse.bass_utils import run_bass_kernel_spmd

    nc = _get_nc(fd_total, fd_tile, **flags)
    shards = prep_input(pred, fd_total)
    in_maps = [{"x": shards[i]} for i in range(_NCORES)]
    return run_bass_kernel_spmd(
        nc, in_maps, list(range(_NCORES)), trace=trace
    )


def combine(results, n_rows_total, n_ycols=8):
    """Host-side: combine per-core partials into the scalar loss (float64)."""
    n_y = n_ycols
    n_rs = 11 - n_y
    n_st = n_y + 2 * n_rs
    g = np.zeros((8 * n_st, 72), np.float64)
    cs = np.zeros(512, np.float64)
    for r in results:
        g += np.asarray(r["gram"], np.float64)
        cs += np.asarray(r["colsum"], np.float64).reshape(-1)

    B = float(n_rows_total)
    # total (raw colsums) per permuted position: x position within a
    # 128-block is col*8 + group, psum col = position mod 512
    k = np.arange(512)
    total_p = np.array([cs[(k % 128) // 8 == c].sum() for c in range(16)])

    # gram layout: stationary row = (slot-1)*8 + group (slots 1..n_st),
    # moving col = slot*8 + group (slots 0..8, slot 0 = ones)
    # pos sums per permuted position 0..10
    pos_p = np.zeros(11)
    for pc in range(n_y):        # y-cols at stationary slot pc+1
        pos_p[pc] = sum(g[pc * 8 + gg, 0] for gg in range(8))
    for i in range(n_rs):        # i-only cols via r/s pairs
        rsum = sum(g[(n_y + i) * 8 + gg, 0] for gg in range(8))
        ssum = sum(g[(n_y + n_rs + i) * 8 + gg, 0] for gg in range(8))
        pos_p[n_y + i] = rsum + 0.25 * ssum + 0.25 * B

    def pp_sum(ci, cj):
        pi, pj = _PC[ci], _PC[cj]
        assert pj < 8
        mv = lambda gg: (pj + 1) * 8 + gg
        if pi < n_y:
            return sum(g[pi * 8 + gg, mv(gg)] for gg in range(8))
        i = pi - n_y
        s = sum(g[(n_y + i) * 8 + gg, mv(gg)]
                + 0.25 * g[(n_y + n_rs + i) * 8 + gg, mv(gg)]
                for gg in range(8))
        return s + 0.25 * pos_p[pj]

    inv_n = 1.0 / B
    pos = {c: pos_p[_PC[c]] * inv_n for c in _JCOLS + _ICOLS}
    neg = {c: (total_p[_PC[c]] - pos_p[_PC[c]]) * inv_n for c in _JCOLS + _ICOLS}

    clamp = lambda v: max(v, 0.0)
    loss = 0.0
    for i, j in _POS_PAIRS:
        pp = pp_sum(i, j) * inv_n
        loss += clamp(pos[i] * pos[j] - pp)
        loss += clamp(neg[i] * pos[j] - pp)
        loss += clamp(pos[i] * neg[j] - pp)
    for i, j in _NEG_PAIRS:
        pp = pp_sum(i, j) * inv_n
        loss += clamp(pos[i] * pos[j] - pp)
        loss += clamp(pp - neg[i] * pos[j])
        loss += clamp(pp - pos[i] * neg[j])
    return loss


def _loss_numpy(pred):
    """CPU fallback: same loss in numpy (used only if the device path fails)."""
    x = pred.astype(np.float64)
    y = np.where(x >= 0.5, x, 0.0)
    n = x.shape[0]
    pos = y.sum(0) / n
    neg = np.where(x < 0.5, x, 0.0).sum(0) / n
    gram = (y.T @ y) / n
    clamp = lambda v: max(v, 0.0)
    loss = 0.0
    for i, j in _POS_PAIRS:
        pp = gram[i, j]
        loss += clamp(pos[i] * pos[j] - pp)
        loss += clamp(neg[i] * pos[j] - pp)
        loss += clamp(pos[i] * neg[j] - pp)
    for i, j in _NEG_PAIRS:
        pp = gram[i, j]
        loss += clamp(pos[i] * pos[j] - pp)
        loss += clamp(pp - neg[i] * pos[j])
        loss += clamp(pp - pos[i] * neg[j])
    return loss


last_path = None  # "device" or "cpu-fallback" — which path the last call took


def kernel(pred, target=None, **_unused):
    global last_path
    import sys
    import traceback
    pred = np.asarray(pred, dtype=np.float32)
    assert pred.shape == (_B, _C), pred.shape
    loss = None
    for backoff in (5.0, 20.0, None):
        try:
            res = run_cores(pred, n_ycols=_N_YCOLS, **_SHIP_FLAGS)
            loss = combine(res.results, _B, n_ycols=_N_YCOLS)
            last_path = "device"
            break
        except Exception:
            # transient device outages usually clear within seconds; fall
            # back to a CPU computation of the identical loss if not
            traceback.print_exc(file=sys.stderr)
            if backoff is not None:
                import time
                time.sleep(backoff)
    if loss is None:
        print("kernel: DEVICE PATH FAILED, using CPU fallback", file=sys.stderr)
        last_path = "cpu-fallback"
        loss = _loss_numpy(pred)
    return np.float32(loss)


# revision 46
# speedup vs baseline: 1.2314x; 1.2314x over previous
"""Trainium2 Bass kernel for nn_Expression_Independent_AU_Loss.

Loss over pred [B=4194304, C=16] (target is unused by the reference):
  pos[c]  = sum_r pred[r,c] * (pred[r,c] >= 0.5) / B
  neg[c]  = sum_r pred[r,c] * (pred[r,c] <  0.5) / B   (= total[c]/B - pos[c])
  pp[i,j] = sum_r y[r,i]*y[r,j] / B   with y = pred * (pred >= 0.5)
followed by a tiny clamp/combine over 14 column pairs (11 distinct columns).

v2 strategy (fp8 + DoubleRow, data-parallel over batch, 8 cores):
  - Host: permute columns so the 8 distinct pair-j columns sit at positions
    0..7 and the 3 i-only columns at 8..10, cast to fp8e4m3 with exact-mask
    repair (values that round up across 0.5 get nudged to 0.46875, the
    largest fp8 below 0.5). DMA halves again vs fp16: 8 MiB/core.
  - Each core views its shard as [128, 65536] fp8; a row-group of 16
    consecutive elements is one full (permuted) row.
  - Per tile: DVE+Pool split-compute y = (x>=0.5)*x for the 8 j-columns into
    a 16-slot z-tile (slot 0 = constant 1). The Act engine computes
    r = Relu(x-0.5) and s = Sign(x-0.484375) for the 3 i-only columns
    (slots 9..11 / 12..14): y = r + s/4 + 1/4 EXACTLY for fp8 inputs, so
    those columns never need the elementwise mask.
  - TensorE (fp8 DoubleRow, 2x MACs, pairs = tile halves):
      gram psum[112,72] += Zst^T @ Zmv  per 8-group chunk
        Zst = slots 1..14 (8 y + 3 r + 3 s), Zmv = slots 0..8 (ones + 8 y)
      colsum psum[1,512] += 1^T @ X     on the raw fp8 tile (total, unmasked)
  - Host combines the tiny per-core partials: reconstructs pp for pairs with
    i-only stationary via pp = G_ry + G_sy/4 + pos_j/4, then clamp/combine.
"""

import numpy as np

_B, _C = 4194304, 16
_NCORES = 8
_FD_TOTAL = _B // _NCORES * _C // 128  # 65536 fp8 per partition per core
_FD_TILE = 8192

_POS_PAIRS = [(0, 1), (2, 5), (2, 6), (5, 6), (4, 8), (6, 11), (9, 11), (9, 14), (11, 14), (13, 14)]
_NEG_PAIRS = [(1, 4), (1, 5), (8, 9), (8, 11)]

# column permutation: j-set first (moving side), then i-only, then unused
_JCOLS = [1, 4, 5, 6, 8, 9, 11, 14]
_ICOLS = [0, 2, 13]
_PERM = _JCOLS + _ICOLS + [3, 7, 10, 12, 15]
_PC = {c: i for i, c in enumerate(_PERM)}  # orig col -> permuted position

_NMV = 9    # moving slots 0..8
_NG = 8     # row-groups per chunk
_N_YCOLS = 8  # cols masked on DVE (rest via act r/s) — tuned empirically
_SHIP_FLAGS = dict(xin_bufs=4, z_bufs=3)  # shipped scheduling config half

_built = {}


def _build(fd_total, fd_tile, repeat=1, xin_bufs=3, mask_frac=1.0,
           n_ycols=8, dve_split=False, relu_on_dve=False, alt_dma=False,
           z_bufs=2, do_mask=True, do_act=True, do_gram=True, do_xsum=True,
           do_dve=True, do_pool=False, do_mm=True):
    """Build + compile the SPMD Bass program for one core shard
    [128, fd_total] fp8e4m3 (flat, contiguous per partition).

    repeat>1 re-runs the whole pass over the same input (differential HW
    timing); partials come out scaled by `repeat`. do_* flags ablate stages
    (timing only). mask_frac = fraction of j-column mask work on DVE
    (rest on Pool/GpSimd).
    """
    import concourse.bass as bass  # noqa: F401
    import concourse.tile as tile
    from concourse import bacc, mybir

    f32 = mybir.dt.float32
    f8 = mybir.dt.float8e4
    DR = mybir.MatmulPerfMode.DoubleRow
    AF = mybir.ActivationFunctionType

    n_tiles = fd_total // fd_tile
    assert fd_total % fd_tile == 0 and fd_tile % 1024 == 0
    n_y = n_ycols               # cols masked on DVE (slots 1..n_y)
    n_rs = 11 - n_y             # i-only cols via act r/s pairs
    n_st = n_y + 2 * n_rs       # stationary slots (1..n_st)
    if n_rs == 0:
        do_act = False

    nc = bacc.Bacc("TRN2", target_bir_lowering=False, debug=False)
    x = nc.dram_tensor("x", [128 * fd_total], f8, kind="ExternalInput").ap()
    gram_out = nc.dram_tensor(
        "gram", [8 * n_st, 72], f32, kind="ExternalOutput").ap()
    colsum_out = nc.dram_tensor("colsum", [1, 512], f32, kind="ExternalOutput").ap()

    G = fd_tile // 16           # row-groups per tile
    NC = G // _NG               # 8-group chunks per tile (chunk-major z)
    n_gram = NC // 2            # DR gram matmuls per tile (chunk c + c+NC/2)
    n_x512 = fd_tile // 1024    # xsum DR matmuls per tile (512 out cols each)

    with tile.TileContext(nc) as tc:
        with (
            tc.tile_pool(name="xin", bufs=xin_bufs) as xin_pool,
            tc.tile_pool(name="zp", bufs=1) as z_pool,
            tc.tile_pool(name="cst", bufs=1) as cst_pool,
            tc.tile_pool(name="outs", bufs=1) as out_pool,
            tc.tile_pool(name="psum", bufs=1, space="PSUM") as psum_pool,
        ):
            if do_xsum:
                # dual-fp8 ldweights needs pair stride >= 64: [128,128] ones
                ones_x = cst_pool.tile([128, 128], f8, tag="onesx")
                nc.vector.memset(ones_x[:], 1.0)
                st_ones = ones_x[:].rearrange("p (two m) -> p two m", two=2)[:, :, 0:1]
                psum_b = psum_pool.tile([128, 512], f32, tag="pb")

            if do_gram:
                psum_a = psum_pool.tile([8 * n_st, 72], f32, tag="pa")
                zbufs = []
                for zi in range(z_bufs):
                    # chunk-major, slot-major: z[p, chunk, slot(16), group(8)]
                    zt = z_pool.tile([128, G * 16], f8, tag=f"z{zi}")
                    z3 = zt[:].rearrange("p (c w) -> p c w", w=128)
                    nc.vector.memset(z3[:, :, 0:8], 1.0)  # slot 0 = ones
                    zbufs.append(zt)
                if do_act:
                    bias_r = cst_pool.tile([128, 1], f32, tag="br")
                    bias_s = cst_pool.tile([128, 1], f32, tag="bs")
                    nc.vector.memset(bias_r[:], -0.5)
                    nc.vector.memset(bias_s[:], -0.484375)

            # chunk split for the DVE/Pool mask work
            s_dve = int(round(mask_frac * NC))

            for r in range(repeat):
                first_r, last_r = r == 0, r == repeat - 1
                off = 0
                for t in range(n_tiles):
                    first_t, last_t = t == 0, t == n_tiles - 1
                    xt = xin_pool.tile([128, fd_tile], f8, tag="x")
                    src = x[128 * off: 128 * (off + fd_tile)].rearrange(
                        "(p f) -> p f", p=128)
                    dma_eng = nc.gpsimd if (alt_dma and t % 2) else nc.sync
                    dma_eng.dma_start(xt[:], src)

                    # x and z share chunk-major blocks of 128: x block =
                    # [col(16) x group(8)] (host pre-transposed), z block =
                    # [slot(16) x group(8)]; all elementwise ops are 3D
                    # stride-1 views [p, chunk, span]
                    xc = xt[:].rearrange("p (c w) -> p c w", w=128)
                    if do_gram:
                        zt = zbufs[t % z_bufs]
                        zc3 = zt[:].rearrange("p (c w) -> p c w", w=128)
                        if do_mask:
                            # y = (x >= 0.5) * x for the first n_y cols
                            # (slots 1..n_y <- cols 0..n_y-1, same group order)
                            if do_dve and s_dve > 0:
                                spans = ([(0, 8)] if n_y == 8 else
                                         [(0, 8), (8, n_y)] if dve_split else
                                         [(0, n_y)])
                                for c0, c1 in spans:
                                    nc.vector.scalar_tensor_tensor(
                                        zc3[:, 0:s_dve, 8 + 8 * c0:8 + 8 * c1],
                                        xc[:, 0:s_dve, 8 * c0:8 * c1],
                                        0.5, xc[:, 0:s_dve, 8 * c0:8 * c1],
                                        op0=mybir.AluOpType.is_ge,
                                        op1=mybir.AluOpType.mult)
                            if do_pool and s_dve < NC:
                                # Pool lacks TensorScalarPtr: two-stage
                                # m = (x>=0.5) in place, then y = m*x
                                nc.gpsimd.tensor_scalar(
                                    zc3[:, s_dve:NC, 8:8 + 8 * n_y],
                                    xc[:, s_dve:NC, 0:8 * n_y],
                                    0.5, None, op0=mybir.AluOpType.is_ge)
                                nc.gpsimd.tensor_tensor(
                                    zc3[:, s_dve:NC, 8:8 + 8 * n_y],
                                    zc3[:, s_dve:NC, 8:8 + 8 * n_y],
                                    xc[:, s_dve:NC, 0:8 * n_y],
                                    op=mybir.AluOpType.mult)
                        if do_act:
                            # r = Relu(x - 0.5), s = Sign(x - 0.484375) for
                            # the n_rs i-only cols; y = r + s/4 + 1/4 exactly
                            ro = 8 * (1 + n_y)
                            so = ro + 8 * n_rs
                            if relu_on_dve:
                                # r = max(x - 0.5, 0) as a DVE tensor_scalar
                                nc.vector.tensor_scalar(
                                    zc3[:, :, ro:ro + 8 * n_rs],
                                    xc[:, :, 8 * n_y:88],
                                    0.5, 0.0,
                                    op0=mybir.AluOpType.subtract,
                                    op1=mybir.AluOpType.max)
                            else:
                                nc.scalar.activation(
                                    zc3[:, :, ro:ro + 8 * n_rs],
                                    xc[:, :, 8 * n_y:88],
                                    AF.Relu, bias=bias_r[:], scale=1.0)
                            nc.scalar.activation(
                                zc3[:, :, so:so + 8 * n_rs],
                                xc[:, :, 8 * n_y:88],
                                AF.Sign, bias=bias_s[:], scale=1.0)

                        # DR gram: pair chunk k with chunk k + NC/2; both
                        # operands contiguous within a chunk block
                        if do_mm:
                            zc = zt[:].rearrange("p (h rest) -> p h rest", h=2)
                            for k in range(n_gram):
                                st = zc[:, :, 128 * k + 8:
                                        128 * k + 8 + 8 * n_st]
                                mv = zc[:, :, 128 * k: 128 * k + 72]
                                nc.tensor.matmul(
                                    psum_a[:, :], st, mv,
                                    start=(first_r and first_t and k == 0),
                                    stop=(last_r and last_t and k == n_gram - 1),
                                    perf_mode=DR)

                    if do_xsum:
                        x2 = xt[:].rearrange("p (h f) -> p h f", h=2)
                        for j in range(n_x512):
                            nc.tensor.matmul(
                                psum_b[0:1, 0:512],
                                st_ones,
                                x2[:, :, 512 * j: 512 * (j + 1)],
                                start=(first_r and first_t and j == 0),
                                stop=(last_r and last_t and j == n_x512 - 1),
                                perf_mode=DR)
                    off += fd_tile

            if do_xsum:
                out_b = out_pool.tile([1, 512], f32, tag="ob")
                nc.vector.tensor_copy(out_b[:], psum_b[0:1, :])
                nc.sync.dma_start(colsum_out[:], out_b[:])
            if do_gram and do_mm:
                out_a = out_pool.tile([8 * n_st, 72], f32, tag="oa")
                nc.vector.tensor_copy(out_a[:], psum_a[:])
                nc.sync.dma_start(gram_out[:], out_a[:])

    nc.compile()
    return nc


def _get_nc(fd_total, fd_tile, repeat=1, xin_bufs=3, **flags):
    key = (fd_total, fd_tile, repeat, xin_bufs, tuple(sorted(flags.items())))
    if key not in _built:
        _built[key] = _build(fd_total, fd_tile, repeat, xin_bufs, **flags)
    return _built[key]


def prep_input(pred, fd_total=_FD_TOTAL):
    """fp8e4m3 staging: permute columns (j-set first), round-nearest cast,
    then nudge elements that rounded up across the 0.5 threshold down to
    0.46875 (largest fp8 < 0.5). Mask becomes exact; value rounding stays
    unbiased apart from the small nudge population (~1.6% of rows)."""
    from concourse import mybir
    np8 = mybir.dt.np(mybir.dt.float8e4)
    p32 = np.ascontiguousarray(pred, dtype=np.float32)[:, _PERM]
    p8 = p32.astype(np8)
    flipped = (p8.astype(np.float32) >= 0.5) & (p32 < 0.5)
    p8[flipped] = np8(0.46875)
    # per-partition chunk blocks of 8 rows transposed to [col(16), group(8)]
    # so every device elementwise op is a 3D stride-1 view
    rows_pp = fd_total // 16
    a = p8.reshape(_NCORES, 128, rows_pp // 8, 8, 16)
    a = np.ascontiguousarray(a.transpose(0, 1, 2, 4, 3))
    return a.reshape(_NCORES, 128 * fd_total)


def run_cores(pred, fd_total=_FD_TOTAL, fd_tile=_FD_TILE, trace=False, **flags):
    """Run the per-core program over all 8 shards; returns raw results."""
    from concourse.bass_utils import run_bass_kernel_spmd

    nc = _get_nc(fd_total, fd_tile, **flags)
    shards = prep_input(pred, fd_total)
    in_maps = [{"x": shards[i]} for i in range(_NCORES)]
    return run_bass_kernel_spmd(
        nc, in_maps, list(range(_NCORES)), trace=trace
    )


def combine(results, n_rows_total, n_ycols=8):
    """Host-side: combine per-core partials into the scalar loss (float64)."""
    n_y = n_ycols
    n_rs = 11 - n_y
    n_st = n_y + 2 * n_rs
    g = np.zeros((8 * n_st, 72), np.float64)
    cs = np.zeros(512, np.float64)
    for r in results:
        g += np.asarray(r["gram"], np.float64)
        cs += np.asarray(r["colsum"], np.float64).reshape(-1)

    B = float(n_rows_total)
    # total (raw colsums) per permuted position: x position within a
    # 128-block is col*8 + group, psum col = position mod 512
    k = np.arange(512)
    total_p = np.array([cs[(k % 128) // 8 == c].sum() for c in range(16)])

    # gram layout: stationary row = (slot-1)*8 + group (slots 1..n_st),
    # moving col = slot*8 + group (slots 0..8, slot 0 = ones)
    # pos sums per permuted position 0..10
    pos_p = np.zeros(11)
    for pc in range(n_y):        # y-cols at stationary slot pc+1
        pos_p[pc] = sum(g[pc * 8 + gg, 0] for gg in range(8))
    for i in range(n_rs):        # i-only cols via r/s pairs
        rsum = sum(g[(n_y + i) * 8 + gg, 0] for gg in range(8))
        ssum = sum(g[(n_y + n_rs + i) * 8 + gg, 0] for gg in range(8))
        pos_p[n_y + i] = rsum + 0.25 * ssum + 0.25 * B

    def pp_sum(ci, cj):
        pi, pj = _PC[ci], _PC[cj]
        assert pj < 8
        mv = lambda gg: (pj + 1) * 8 + gg
        if pi < n_y:
            return sum(g[pi * 8 + gg, mv(gg)] for gg in range(8))
        i = pi - n_y
        s = sum(g[(n_y + i) * 8 + gg, mv(gg)]
                + 0.25 * g[(n_y + n_rs + i) * 8 + gg, mv(gg)]
                for gg in range(8))
        return s + 0.25 * pos_p[pj]

    inv_n = 1.0 / B
    pos = {c: pos_p[_PC[c]] * inv_n for c in _JCOLS + _ICOLS}
    neg = {c: (total_p[_PC[c]] - pos_p[_PC[c]]) * inv_n for c in _JCOLS + _ICOLS}

    clamp = lambda v: max(v, 0.0)
    loss = 0.0
    for i, j in _POS_PAIRS:
        pp = pp_sum(i, j) * inv_n
        loss += clamp(pos[i] * pos[j] - pp)
        loss += clamp(neg[i] * pos[j] - pp)
        loss += clamp(pos[i] * neg[j] - pp)
    for i, j in _NEG_PAIRS:
        pp = pp_sum(i, j) * inv_n
        loss += clamp(pos[i] * pos[j] - pp)
        loss += clamp(pp - neg[i] * pos[j])
        loss += clamp(pp - pos[i] * neg[j])
    return loss


def _loss_numpy(pred):
    """CPU fallback: same loss in numpy (used only if the device path fails)."""
    x = pred.astype(np.float64)
    y = np.where(x >= 0.5, x, 0.0)
    n = x.shape[0]
    pos = y.sum(0) / n
    neg = np.where(x < 0.5, x, 0.0).sum(0) / n
    gram = (y.T @ y) / n
    clamp = lambda v: max(v, 0.0)
    loss = 0.0
    for i, j in _POS_PAIRS:
        pp = gram[i, j]
        loss += clamp(pos[i] * pos[j] - pp)
        loss += clamp(neg[i] * pos[j] - pp)
        loss += clamp(pos[i] * neg[j] - pp)
    for i, j in _NEG_PAIRS:
        pp = gram[i, j]
        loss += clamp(pos[i] * pos[j] - pp)
        loss += clamp(pp - neg[i] * pos[j])
        loss += clamp(pp - pos[i] * neg[j])
    return loss


last_path = None  # "device" or "cpu-fallback" — which path the last call took


def kernel(pred, target=None, **_unused):
    global last_path
    import sys
    import traceback
    pred = np.asarray(pred, dtype=np.float32)
    assert pred.shape == (_B, _C), pred.shape
    loss = None
    for backoff in (5.0, 20.0, None):
        try:
            res = run_cores(pred, n_ycols=_N_YCOLS, **_SHIP_FLAGS)
            loss = combine(res.results, _B, n_ycols=_N_YCOLS)
            last_path = "device"
            break
        except Exception:
            # transient device outages usually clear within seconds; fall
            # back to a CPU computation of the identical loss if not
            traceback.print_exc(file=sys.stderr)
            if backoff is not None:
                import time
                time.sleep(backoff)
    if loss is None:
        print("kernel: DEVICE PATH FAILED, using CPU fallback", file=sys.stderr)
        last_path = "cpu-fallback"
        loss = _loss_numpy(pred)
    return np.float32(loss)


# revision 51
# speedup vs baseline: 2.4041x; 1.9523x over previous
"""Trainium2 Bass kernel for nn_Expression_Independent_AU_Loss.

Loss over pred [B=4194304, C=16] (target is unused by the reference):
  pos[c]  = sum_r pred[r,c] * (pred[r,c] >= 0.5) / B
  neg[c]  = sum_r pred[r,c] * (pred[r,c] <  0.5) / B   (= total[c]/B - pos[c])
  pp[i,j] = sum_r y[r,i]*y[r,j] / B   with y = pred * (pred >= 0.5)
followed by a tiny clamp/combine over 14 column pairs (11 distinct columns).

v2 strategy (fp8 + DoubleRow, data-parallel over batch, 8 cores):
  - Host: permute columns so the 8 distinct pair-j columns sit at positions
    0..7 and the 3 i-only columns at 8..10, cast to fp8e4m3 with exact-mask
    repair (values that round up across 0.5 get nudged to 0.46875, the
    largest fp8 below 0.5). DMA halves again vs fp16: 8 MiB/core.
  - Each core views its shard as [128, 65536] fp8; a row-group of 16
    consecutive elements is one full (permuted) row.
  - Per tile: DVE+Pool split-compute y = (x>=0.5)*x for the 8 j-columns into
    a 16-slot z-tile (slot 0 = constant 1). The Act engine computes
    r = Relu(x-0.5) and s = Sign(x-0.484375) for the 3 i-only columns
    (slots 9..11 / 12..14): y = r + s/4 + 1/4 EXACTLY for fp8 inputs, so
    those columns never need the elementwise mask.
  - TensorE (fp8 DoubleRow, 2x MACs, pairs = tile halves):
      gram psum[112,72] += Zst^T @ Zmv  per 8-group chunk
        Zst = slots 1..14 (8 y + 3 r + 3 s), Zmv = slots 0..8 (ones + 8 y)
      colsum psum[1,512] += 1^T @ X     on the raw fp8 tile (total, unmasked)
  - Host combines the tiny per-core partials: reconstructs pp for pairs with
    i-only stationary via pp = G_ry + G_sy/4 + pos_j/4, then clamp/combine.
"""

import numpy as np

_B, _C = 4194304, 16
_NCORES = 8
_FD_TOTAL = _B // _NCORES * _C // 128  # 65536 fp8 per partition per core
_FD_TILE = 8192

_POS_PAIRS = [(0, 1), (2, 5), (2, 6), (5, 6), (4, 8), (6, 11), (9, 11), (9, 14), (11, 14), (13, 14)]
_NEG_PAIRS = [(1, 4), (1, 5), (8, 9), (8, 11)]

# column permutation: j-set first (moving side), then i-only, then unused
_JCOLS = [1, 4, 5, 6, 8, 9, 11, 14]
_ICOLS = [0, 2, 13]
_PERM = _JCOLS + _ICOLS + [3, 7, 10, 12, 15]
_PC = {c: i for i, c in enumerate(_PERM)}  # orig col -> permuted position

_NMV = 9    # moving slots 0..8
_NG = 8     # row-groups per chunk
_N_YCOLS = 8  # cols masked on DVE (rest via act r/s) — tuned empirically
_SHIP_FLAGS = dict(xin_bufs=4, z_bufs=3)  # shipped scheduling config half

_built = {}


def _build(fd_total, fd_tile, repeat=1, xin_bufs=3, mask_frac=1.0,
           n_ycols=8, dve_split=False, relu_on_dve=False, alt_dma=False,
           z_bufs=2, do_mask=True, do_act=True, do_gram=True, do_xsum=True,
           do_dve=True, do_pool=False, do_mm=True):
    """Build + compile the SPMD Bass program for one core shard
    [128, fd_total] fp8e4m3 (flat, contiguous per partition).

    repeat>1 re-runs the whole pass over the same input (differential HW
    timing); partials come out scaled by `repeat`. do_* flags ablate stages
    (timing only). mask_frac = fraction of j-column mask work on DVE
    (rest on Pool/GpSimd).
    """
    import concourse.bass as bass  # noqa: F401
    import concourse.tile as tile
    from concourse import bacc, mybir

    f32 = mybir.dt.float32
    f8 = mybir.dt.float8e4
    DR = mybir.MatmulPerfMode.DoubleRow
    AF = mybir.ActivationFunctionType

    n_tiles = fd_total // fd_tile
    assert fd_total % fd_tile == 0 and fd_tile % 1024 == 0
    # taper: shrink the final tiles so the serialized post-last-DMA tail
    # (mask + act + gram + evac of the last tile) is a fraction of a full
    # tile's latency
    if n_tiles >= 4 and fd_tile >= 4096:
        sizes = [fd_tile] * (n_tiles - 1) + [fd_tile // 2, fd_tile // 4,
                                             fd_tile // 4]
    else:
        sizes = [fd_tile] * n_tiles
    assert sum(sizes) == fd_total and all(s % 1024 == 0 for s in sizes)
    n_y = n_ycols               # cols masked on DVE (slots 1..n_y)
    n_rs = 11 - n_y             # i-only cols via act r/s pairs
    n_st = n_y + 2 * n_rs       # stationary slots (1..n_st)
    if n_rs == 0:
        do_act = False

    nc = bacc.Bacc("TRN2", target_bir_lowering=False, debug=False)
    x = nc.dram_tensor("x", [128 * fd_total], f8, kind="ExternalInput").ap()
    gram_out = nc.dram_tensor(
        "gram", [8 * n_st, 72], f32, kind="ExternalOutput").ap()
    colsum_out = nc.dram_tensor("colsum", [1, 512], f32, kind="ExternalOutput").ap()

    G = fd_tile // 16           # row-groups per tile
    NC = G // _NG               # 8-group chunks per tile (chunk-major z)
    n_gram = NC // 2            # DR gram matmuls per tile (chunk c + c+NC/2)
    n_x512 = fd_tile // 1024    # xsum DR matmuls per tile (512 out cols each)

    with tile.TileContext(nc) as tc:
        with (
            tc.tile_pool(name="xin", bufs=xin_bufs) as xin_pool,
            tc.tile_pool(name="zp", bufs=1) as z_pool,
            tc.tile_pool(name="cst", bufs=1) as cst_pool,
            tc.tile_pool(name="outs", bufs=1) as out_pool,
            tc.tile_pool(name="psum", bufs=1, space="PSUM") as psum_pool,
        ):
            if do_xsum:
                # dual-fp8 ldweights needs pair stride >= 64: [128,128] ones
                ones_x = cst_pool.tile([128, 128], f8, tag="onesx")
                nc.vector.memset(ones_x[:], 1.0)
                st_ones = ones_x[:].rearrange("p (two m) -> p two m", two=2)[:, :, 0:1]
                psum_b = psum_pool.tile([128, 512], f32, tag="pb")

            if do_gram:
                psum_a = psum_pool.tile([8 * n_st, 72], f32, tag="pa")
                zbufs = []
                for zi in range(z_bufs):
                    # chunk-major, slot-major: z[p, chunk, slot(16), group(8)]
                    zt = z_pool.tile([128, G * 16], f8, tag=f"z{zi}")
                    z3 = zt[:].rearrange("p (c w) -> p c w", w=128)
                    nc.vector.memset(z3[:, :, 0:8], 1.0)  # slot 0 = ones
                    zbufs.append(zt)
                if do_act:
                    bias_r = cst_pool.tile([128, 1], f32, tag="br")
                    bias_s = cst_pool.tile([128, 1], f32, tag="bs")
                    nc.vector.memset(bias_r[:], -0.5)
                    nc.vector.memset(bias_s[:], -0.484375)

            for r in range(repeat):
                first_r, last_r = r == 0, r == repeat - 1
                off = 0
                for t, fsz in enumerate(sizes):
                    first_t, last_t = t == 0, t == len(sizes) - 1
                    NC = fsz // 128
                    n_gram = NC // 2
                    n_x512 = fsz // 1024
                    s_dve = int(round(mask_frac * NC))
                    xt = xin_pool.tile([128, fsz], f8, tag="x")
                    src = x[128 * off: 128 * (off + fsz)].rearrange(
                        "(p f) -> p f", p=128)
                    dma_eng = nc.gpsimd if (alt_dma and t % 2) else nc.sync
                    dma_eng.dma_start(xt[:], src)

                    # x and z share chunk-major blocks of 128: x block =
                    # [col(16) x group(8)] (host pre-transposed), z block =
                    # [slot(16) x group(8)]; all elementwise ops are 3D
                    # stride-1 views [p, chunk, span]
                    xc = xt[:].rearrange("p (c w) -> p c w", w=128)
                    if do_gram:
                        zt = zbufs[t % z_bufs]
                        zc3 = zt[:].rearrange("p (c w) -> p c w", w=128)
                        if do_mask:
                            # y = (x >= 0.5) * x for the first n_y cols
                            # (slots 1..n_y <- cols 0..n_y-1, same group order)
                            if do_dve and s_dve > 0:
                                spans = ([(0, 8)] if n_y == 8 else
                                         [(0, 8), (8, n_y)] if dve_split else
                                         [(0, n_y)])
                                for c0, c1 in spans:
                                    nc.vector.scalar_tensor_tensor(
                                        zc3[:, 0:s_dve, 8 + 8 * c0:8 + 8 * c1],
                                        xc[:, 0:s_dve, 8 * c0:8 * c1],
                                        0.5, xc[:, 0:s_dve, 8 * c0:8 * c1],
                                        op0=mybir.AluOpType.is_ge,
                                        op1=mybir.AluOpType.mult)
                            if do_pool and s_dve < NC:
                                # Pool lacks TensorScalarPtr: two-stage
                                # m = (x>=0.5) in place, then y = m*x
                                nc.gpsimd.tensor_scalar(
                                    zc3[:, s_dve:NC, 8:8 + 8 * n_y],
                                    xc[:, s_dve:NC, 0:8 * n_y],
                                    0.5, None, op0=mybir.AluOpType.is_ge)
                                nc.gpsimd.tensor_tensor(
                                    zc3[:, s_dve:NC, 8:8 + 8 * n_y],
                                    zc3[:, s_dve:NC, 8:8 + 8 * n_y],
                                    xc[:, s_dve:NC, 0:8 * n_y],
                                    op=mybir.AluOpType.mult)
                        if do_act:
                            # r = Relu(x - 0.5), s = Sign(x - 0.484375) for
                            # the n_rs i-only cols; y = r + s/4 + 1/4 exactly
                            ro = 8 * (1 + n_y)
                            so = ro + 8 * n_rs
                            if relu_on_dve:
                                # r = max(x - 0.5, 0) as a DVE tensor_scalar
                                nc.vector.tensor_scalar(
                                    zc3[:, 0:NC, ro:ro + 8 * n_rs],
                                    xc[:, :, 8 * n_y:88],
                                    0.5, 0.0,
                                    op0=mybir.AluOpType.subtract,
                                    op1=mybir.AluOpType.max)
                            else:
                                nc.scalar.activation(
                                    zc3[:, 0:NC, ro:ro + 8 * n_rs],
                                    xc[:, :, 8 * n_y:88],
                                    AF.Relu, bias=bias_r[:], scale=1.0)
                            nc.scalar.activation(
                                zc3[:, 0:NC, so:so + 8 * n_rs],
                                xc[:, :, 8 * n_y:88],
                                AF.Sign, bias=bias_s[:], scale=1.0)

                        # DR gram: pair chunk k with chunk k + NC/2; both
                        # operands contiguous within a chunk block
                        if do_mm:
                            zc = zt[:, 0:fsz].rearrange(
                                "p (h rest) -> p h rest", h=2)
                            for k in range(n_gram):
                                st = zc[:, :, 128 * k + 8:
                                        128 * k + 8 + 8 * n_st]
                                mv = zc[:, :, 128 * k: 128 * k + 72]
                                nc.tensor.matmul(
                                    psum_a[:, :], st, mv,
                                    start=(first_r and first_t and k == 0),
                                    stop=(last_r and last_t and k == n_gram - 1),
                                    perf_mode=DR)

                    if do_xsum:
                        x2 = xt[:].rearrange("p (h f) -> p h f", h=2)
                        for j in range(n_x512):
                            nc.tensor.matmul(
                                psum_b[0:1, 0:512],
                                st_ones,
                                x2[:, :, 512 * j: 512 * (j + 1)],
                                start=(first_r and first_t and j == 0),
                                stop=(last_r and last_t and j == n_x512 - 1),
                                perf_mode=DR)
                    off += fsz

            if do_xsum:
                out_b = out_pool.tile([1, 512], f32, tag="ob")
                nc.vector.tensor_copy(out_b[:], psum_b[0:1, :])
                nc.sync.dma_start(colsum_out[:], out_b[:])
            if do_gram and do_mm:
                out_a = out_pool.tile([8 * n_st, 72], f32, tag="oa")
                nc.vector.tensor_copy(out_a[:], psum_a[:])
                nc.sync.dma_start(gram_out[:], out_a[:])

    nc.compile()
    return nc


def _get_nc(fd_total, fd_tile, repeat=1, xin_bufs=3, **flags):
    key = (fd_total, fd_tile, repeat, xin_bufs, tuple(sorted(flags.items())))
    if key not in _built:
        _built[key] = _build(fd_total, fd_tile, repeat, xin_bufs, **flags)
    return _built[key]


def prep_input(pred, fd_total=_FD_TOTAL):
    """fp8e4m3 staging: permute columns (j-set first), round-nearest cast,
    then nudge elements that rounded up across the 0.5 threshold down to
    0.46875 (largest fp8 < 0.5). Mask becomes exact; value rounding stays
    unbiased apart from the small nudge population (~1.6% of rows)."""
    from concourse import mybir
    np8 = mybir.dt.np(mybir.dt.float8e4)
    p32 = np.ascontiguousarray(pred, dtype=np.float32)[:, _PERM]
    p8 = p32.astype(np8)
    flipped = (p8.astype(np.float32) >= 0.5) & (p32 < 0.5)
    p8[flipped] = np8(0.46875)
    # per-partition chunk blocks of 8 rows transposed to [col(16), group(8)]
    # so every device elementwise op is a 3D stride-1 view
    rows_pp = fd_total // 16
    a = p8.reshape(_NCORES, 128, rows_pp // 8, 8, 16)
    a = np.ascontiguousarray(a.transpose(0, 1, 2, 4, 3))
    return a.reshape(_NCORES, 128 * fd_total)


def run_cores(pred, fd_total=_FD_TOTAL, fd_tile=_FD_TILE, trace=False, **flags):
    """Run the per-core program over all 8 shards; returns raw results."""
    from concourse.bass_utils import run_bass_kernel_spmd

    nc = _get_nc(fd_total, fd_tile, **flags)
    shards = prep_input(pred, fd_total)
    in_maps = [{"x": shards[i]} for i in range(_NCORES)]
    return run_bass_kernel_spmd(
        nc, in_maps, list(range(_NCORES)), trace=trace
    )


def combine(results, n_rows_total, n_ycols=8):
    """Host-side: combine per-core partials into the scalar loss (float64)."""
    n_y = n_ycols
    n_rs = 11 - n_y
    n_st = n_y + 2 * n_rs
    g = np.zeros((8 * n_st, 72), np.float64)
    cs = np.zeros(512, np.float64)
    for r in results:
        g += np.asarray(r["gram"], np.float64)
        cs += np.asarray(r["colsum"], np.float64).reshape(-1)

    B = float(n_rows_total)
    # total (raw colsums) per permuted position: x position within a
    # 128-block is col*8 + group, psum col = position mod 512
    k = np.arange(512)
    total_p = np.array([cs[(k % 128) // 8 == c].sum() for c in range(16)])

    # gram layout: stationary row = (slot-1)*8 + group (slots 1..n_st),
    # moving col = slot*8 + group (slots 0..8, slot 0 = ones)
    # pos sums per permuted position 0..10
    pos_p = np.zeros(11)
    for pc in range(n_y):        # y-cols at stationary slot pc+1
        pos_p[pc] = sum(g[pc * 8 + gg, 0] for gg in range(8))
    for i in range(n_rs):        # i-only cols via r/s pairs
        rsum = sum(g[(n_y + i) * 8 + gg, 0] for gg in range(8))
        ssum = sum(g[(n_y + n_rs + i) * 8 + gg, 0] for gg in range(8))
        pos_p[n_y + i] = rsum + 0.25 * ssum + 0.25 * B

    def pp_sum(ci, cj):
        pi, pj = _PC[ci], _PC[cj]
        assert pj < 8
        mv = lambda gg: (pj + 1) * 8 + gg
        if pi < n_y:
            return sum(g[pi * 8 + gg, mv(gg)] for gg in range(8))
        i = pi - n_y
        s = sum(g[(n_y + i) * 8 + gg, mv(gg)]
                + 0.25 * g[(n_y + n_rs + i) * 8 + gg, mv(gg)]
                for gg in range(8))
        return s + 0.25 * pos_p[pj]

    inv_n = 1.0 / B
    pos = {c: pos_p[_PC[c]] * inv_n for c in _JCOLS + _ICOLS}
    neg = {c: (total_p[_PC[c]] - pos_p[_PC[c]]) * inv_n for c in _JCOLS + _ICOLS}

    clamp = lambda v: max(v, 0.0)
    loss = 0.0
    for i, j in _POS_PAIRS:
        pp = pp_sum(i, j) * inv_n
        loss += clamp(pos[i] * pos[j] - pp)
        loss += clamp(neg[i] * pos[j] - pp)
        loss += clamp(pos[i] * neg[j] - pp)
    for i, j in _NEG_PAIRS:
        pp = pp_sum(i, j) * inv_n
        loss += clamp(pos[i] * pos[j] - pp)
        loss += clamp(pp - neg[i] * pos[j])
        loss += clamp(pp - pos[i] * neg[j])
    return loss


def _loss_numpy(pred):
    """CPU fallback: same loss in numpy (used only if the device path fails)."""
    x = pred.astype(np.float64)
    y = np.where(x >= 0.5, x, 0.0)
    n = x.shape[0]
    pos = y.sum(0) / n
    neg = np.where(x < 0.5, x, 0.0).sum(0) / n
    gram = (y.T @ y) / n
    clamp = lambda v: max(v, 0.0)
    loss = 0.0
    for i, j in _POS_PAIRS:
        pp = gram[i, j]
        loss += clamp(pos[i] * pos[j] - pp)
        loss += clamp(neg[i] * pos[j] - pp)
        loss += clamp(pos[i] * neg[j] - pp)
    for i, j in _NEG_PAIRS:
        pp = gram[i, j]
        loss += clamp(pos[i] * pos[j] - pp)
        loss += clamp(pp - neg[i] * pos[j])
        loss += clamp(pp - pos[i] * neg[j])
    return loss


last_path = None  # "device" or "cpu-fallback" — which path the last call took


def kernel(pred, target=None, **_unused):
    global last_path
    import sys
    import traceback
    pred = np.asarray(pred, dtype=np.float32)
    assert pred.shape == (_B, _C), pred.shape
    loss = None
    for backoff in (5.0, 20.0, None):
        try:
            res = run_cores(pred, n_ycols=_N_YCOLS, **_SHIP_FLAGS)
            loss = combine(res.results, _B, n_ycols=_N_YCOLS)
            last_path = "device"
            break
        except Exception:
            # transient device outages usually clear within seconds; fall
            # back to a CPU computation of the identical loss if not
            traceback.print_exc(file=sys.stderr)
            if backoff is not None:
                import time
                time.sleep(backoff)
    if loss is None:
        print("kernel: DEVICE PATH FAILED, using CPU fallback", file=sys.stderr)
        last_path = "cpu-fallback"
        loss = _loss_numpy(pred)
    return np.float32(loss)
